# revision 20
# baseline (speedup 1.0000x reference)
"""Attention2D Trainium2 Bass kernel — fp8 DoubleRow, 2-engine softmax.

Reference computation (per batch image, C=512 channels, N=1024 tokens):
    qkv = qkv_w @ x + qkv_b            # (1536, N)
    q,k,v per head (8 heads, head_dim 64)
    attn = softmax(scale * q.T k)      # (N, N) per head, scale = C**-0.5
    out  = v @ attn.T                  # (64, N) per head
    y    = x + proj_w @ out + proj_b

Sharding: data-parallel over batch. 16 images / 8 cores = 2 images per core.
Weights are replicated; no collectives.

All matmuls are fp8e4m3 MatmulPerfMode.DoubleRow (2 k-tiles per instruction,
0.5 PE cycles per output column). End-to-end fp8 gives rel err ~2.7e-3.

The exp(SCALE*S) work (16M elements/core) is the bottleneck; it is split
between two engines:
  - ACT: real exp activation, full [128,1024] slabs, fp8 output.
  - DVE: Schraudolph bit-trick exp on [128,512] half slabs —
    fp8e4m3 bits of e^x are approximately round(x*log2(e)*8 + 56), computed
    as one tensor_scalar (mult, add) with int8 output aliasing the fp8 slab.
    The piecewise-linear error (~3%) is below fp8 quantization noise, and
    any constant multiplicative bias cancels in the softmax normalization.

Softmax denominators come for free: V' tiles carry 64 columns of ones, so
the O matmul produces [128, 512] psum where rows 0..63 are the per-channel
numerators and rows 64..127 are 64 copies of the denominator. One DVE
divide (rows 0:64 by rows 64:128) normalizes and casts to fp8 in a single
op — no reciprocal, no DMA broadcast round trip.

Layouts (t = DoubleRow k-tile index, strides always %16==0):
  - xq  [128, (2 t, 1024 n)] per (img, cc2): x channel c = cc2*256+t*128+p.
  - wqkv[128, (2 t, 1536 o)] per cc2, pw likewise.
  - QK  [128, (4 blk, 1024)] per (img, pair): [Q | zeros | K | zeros];
    S contracts head_dim 64 as 64 partitions x 2 k-tiles, second k-tile
    reading the adjacent zero block.
  - V'  [128 m, (2 t, 8 h, 128)] per (img, mc-pair): v in cols 0..64,
    ones in cols 64..128 (denominator trick).
  - E'  [128 m, (2 t, 1024 n)] fp8 slabs, written by ACT/DVE, DoubleRow rhs
    of the O matmul.
  - ON' [128, (2 t, 1024 n)] per (img, cc2): normalized attention output.
"""

import numpy as np

import concourse.bass as bass
import concourse.tile as tile
from concourse import mybir
from concourse.bass_utils import run_bass_kernel_spmd

B, C, N = 16, 512, 1024
HEADS, HD = 8, 64
SCALE = float(C) ** -0.5
NCORES = 8
BPC = B // NCORES  # images per core
MC = 8             # m-chunks of 128 keys
MM_MODE = "fp8"

F32 = mybir.dt.float32
F32R = mybir.dt.float32r
F8 = mybir.dt.float8e4
I8 = mybir.dt.int8
NP8 = mybir.dt.np(F8)
DRM = mybir.MatmulPerfMode.DoubleRow
ALU = mybir.AluOpType

# Schraudolph fp8-exp constants: bits(e^(SCALE*s)) ~= s*A_S + B_S
A_S = SCALE * np.log2(np.e) * 8.0
B_S = 7.0 * 8.0 - 0.5


def dve_mcs(gh):
    """Which m-chunks of global head gh (0..15) run their exp on DVE.
    ~39 of 128 m-chunks go to DVE to balance ACT (exp) against DVE
    (Schraudolph exp + v-casts + normalize + half the qk casts)."""
    return (3, 5, 7) if gh % 16 < 7 else (3, 7)


def _split_multi_waits(nc):
    """Walrus codegen in this toolchain rejects instructions carrying more
    than one semaphore wait ("Too many sync wait commands"). Hoist all but
    the last wait of such instructions into standalone InstEventSemaphore
    ops just before them (same engine, so per-engine order is preserved)."""
    n_split = 0
    for f in nc.m.functions:
        for b in f.blocks:
            out = []
            changed = False
            for inst in b.instructions:
                si = inst.sync_info
                waits = list(si.on_wait) if si is not None else []
                if len(waits) > 1:
                    for k, w in enumerate(waits[:-1]):
                        wi = mybir.InstEventSemaphore(
                            name=f"{inst.name}-presync{k}", ins=[], outs=[],
                            sync_info=mybir.SyncInfo(on_wait=[w], on_update=[]),
                        )
                        wi.engine = inst.engine
                        out.append(wi)
                        n_split += 1
                    inst.sync_info = mybir.SyncInfo(
                        on_wait=[waits[-1]], on_update=list(si.on_update)
                    )
                    changed = True
                out.append(inst)
            if changed:
                b.instructions = out
    return n_split


def build_nc():
    nc = bass.Bass()
    xq_h = nc.dram_tensor("xq", [BPC, 2, 128, 2 * N], F8, kind="ExternalInput")
    xr_h = nc.dram_tensor("xr", [BPC, C, N], F32R, kind="ExternalInput")
    wqkv_h = nc.dram_tensor("wqkv", [2, 128, 2 * 1536], F8, kind="ExternalInput")
    pw_h = nc.dram_tensor("pw", [2, 128, 2 * 512], F8, kind="ExternalInput")
    eye_h = nc.dram_tensor("eye", [128, 128], F32R, kind="ExternalInput")
    bqk_h = nc.dram_tensor("bqk", [128, 8], F32, kind="ExternalInput")
    bv_h = nc.dram_tensor("bv", [128, C], F32, kind="ExternalInput")
    zz_h = nc.dram_tensor("zz", [1, N], F8, kind="ExternalInput")
    oo_h = nc.dram_tensor("oo", [1, N], F8, kind="ExternalInput")
    y_h = nc.dram_tensor("y", [BPC, C, N], F32, kind="ExternalOutput")

    dma = nc.sync.dma_start

    BUFS = dict(qk=8, xq=4, vt=8, es=7, on=4, xr=6, y=2)

    with tile.TileContext(nc) as tc:
        with (
            tc.tile_pool(name="w", bufs=1) as wp,
            tc.tile_pool(name="sb", bufs=2) as sb,
            tc.tile_pool(name="sa", bufs=2, space=bass.MemorySpace.PSUM) as sa,
            tc.tile_pool(name="sd", bufs=2, space=bass.MemorySpace.PSUM) as sd,
            tc.tile_pool(name="po", bufs=1, space=bass.MemorySpace.PSUM) as po,
            tc.tile_pool(name="mm", bufs=1, space=bass.MemorySpace.PSUM) as mm,
        ):
            wq_sb, pw_sb = [], []
            xq_sb, xr_sb = {}, {}
            qk_sb = {}   # (img, pair) -> [128, 4096] fp8 [Q|Z|K|Z]
            vt_sb = {}   # (img, mcp)  -> [128, 2048] fp8 (2t, 8h, 64v+64ones)
            on_sb = {}   # (img, cc2)  -> [128, 2048] fp8

            def load_weights_and_x0():
                for cc2 in range(2):
                    w = wp.tile([128, 2 * 1536], F8, tag=f"wq{cc2}",
                                name=f"wq{cc2}")
                    wq_sb.append(w)
                    t = sb.tile([128, 2 * N], F8, tag="xq", bufs=BUFS["xq"],
                                name=f"xq0_{cc2}")
                    xq_sb[(0, cc2)] = t
                    weng = nc.scalar if cc2 == 0 else nc.gpsimd
                    xeng = nc.sync if cc2 == 0 else nc.scalar
                    for q in range(4):
                        weng.dma_start(
                            out=w[:, q * 768:(q + 1) * 768],
                            in_=wqkv_h[cc2, :, q * 768:(q + 1) * 768])
                        xeng.dma_start(
                            out=t[:, q * 512:(q + 1) * 512],
                            in_=xq_h[0, cc2, :, q * 512:(q + 1) * 512])
                bqk = wp.tile([128, 8], F32, tag="bqk", name="bqk")
                nc.gpsimd.dma_start(out=bqk[:], in_=bqk_h[:])
                bv = wp.tile([128, C], F32, tag="bv", name="bv")
                nc.gpsimd.dma_start(out=bv[:], in_=bv_h[:])
                eye = wp.tile([128, 128], F32R, tag="eye", name="eye")
                nc.gpsimd.dma_start(out=eye[:], in_=eye_h[:])
                for cc2 in range(2):
                    t = wp.tile([128, 2 * 512], F8, tag=f"pw{cc2}",
                                name=f"pw{cc2}")
                    nc.gpsimd.dma_start(out=t[:], in_=pw_h[cc2])
                    pw_sb.append(t)
                return bqk, bv, eye

            def load_xq(img):
                for cc2 in range(2):
                    t = sb.tile([128, 2 * N], F8, tag="xq", bufs=BUFS["xq"],
                                name=f"xq{img}_{cc2}")
                    dma(out=t[:, 0:N], in_=xq_h[img, cc2, :, 0:N])
                    nc.gpsimd.dma_start(out=t[:, N:2 * N],
                                        in_=xq_h[img, cc2, :, N:2 * N])
                    xq_sb[(img, cc2)] = t

            def load_xr(img):
                for oc in range(4):
                    t = sb.tile([128, N], F32R, tag="xr", bufs=BUFS["xr"],
                                name=f"xr{img}_{oc}")
                    nc.gpsimd.dma_start(
                        out=t[:], in_=xr_h[img, oc * 128:(oc + 1) * 128, :])
                    xr_sb[(img, oc)] = t

            def alloc_qk(img):
                for pair in range(4):
                    t = sb.tile([128, 4096], F8, tag="qk", bufs=BUFS["qk"],
                                name=f"qk{img}_{pair}")
                    qk_sb[(img, pair)] = t
                    dma(out=t[:, N:2 * N],
                        in_=zz_h[:].partition_broadcast(128))
                    dma(out=t[:, 3 * N:4 * N],
                        in_=zz_h[:].partition_broadcast(128))

            def alloc_on(img):
                for cc2 in range(2):
                    on_sb[(img, cc2)] = sb.tile(
                        [128, 2 * N], F8, tag="on", bufs=BUFS["on"],
                        name=f"on{img}_{cc2}")

            def mm_qk(img, oc, nh):
                # q/k block oc (0..3 = q pairs, 4..7 = k pairs), n-half nh.
                # The psum->fp8 bias-add cast alternates DVE/ACT for balance.
                q_ps = mm.tile([128, 512], F32, tag="mm", name=f"q{img}{oc}{nh}")
                for cc2 in range(2):
                    wv = wq_sb[cc2][:].rearrange("p (t o) -> p t o", t=2)
                    xv = xq_sb[(img, cc2)][:].rearrange("p (t n) -> p t n", t=2)
                    nc.tensor.matmul(
                        q_ps[:],
                        wv[:, :, oc * 128:(oc + 1) * 128],
                        xv[:, :, nh * 512:(nh + 1) * 512],
                        start=(cc2 == 0), stop=(cc2 == 1), perf_mode=DRM)
                pair, isk = oc % 4, oc // 4
                dst = qk_sb[(img, pair)][
                    :, isk * 2 * N + nh * 512:isk * 2 * N + nh * 512 + 512]
                if nh == 0:
                    nc.vector.tensor_scalar_add(dst, q_ps[:],
                                                bqk_sb[:, oc:oc + 1])
                else:
                    nc.scalar.activation(
                        dst, q_ps[:], mybir.ActivationFunctionType.Identity,
                        bias=bqk_sb[:, oc:oc + 1])

            def mm_v(img, mc):
                mcp, sub = mc // 2, mc % 2
                if sub == 0:
                    vt = sb.tile([128, 2048], F8, tag="vt", bufs=BUFS["vt"],
                                 name=f"vt{img}_{mcp}")
                    vt_sb[(img, mcp)] = vt
                    tv = vt[:].rearrange("p (t h u) -> p t h u", t=2, u=128)
                    # ones block (denominator trick) via broadcast DMA
                    dma(out=tv[:, :, :, 64:128],
                        in_=oo_h[:, 0:N].rearrange("o (t h u) -> o t h u",
                                                   t=2, u=64)
                        .partition_broadcast(128))
                vt = vt_sb[(img, mcp)]
                tv = vt[:].rearrange("p (t h u) -> p t h u", t=2, u=128)
                v_ps = mm.tile([128, 512], F32, tag="mm", name=f"v{img}{mc}")
                for cc2 in range(2):
                    xv = xq_sb[(img, cc2)][:].rearrange("p (t n) -> p t n", t=2)
                    wv = wq_sb[cc2][:].rearrange("p (t o) -> p t o", t=2)
                    nc.tensor.matmul(
                        v_ps[:],
                        xv[:, :, mc * 128:(mc + 1) * 128],
                        wv[:, :, 1024:1536],
                        start=(cc2 == 0), stop=(cc2 == 1), perf_mode=DRM)
                nc.vector.tensor_add(
                    tv[:, sub, :, 0:64],
                    v_ps[:].rearrange("p (h u) -> p h u", u=64),
                    bv_sb[:].rearrange("p (h u) -> p h u", u=64))

            def mm_proj(img, oc, nh):
                # proj psum accumulates the x+proj_b residual via an extra
                # f32r identity matmul (PE has slack); the psum->sbuf copy
                # then runs on ACT, freeing DVE entirely.
                p_ps = mm.tile([128, 512], F32, tag="mm", name=f"p{img}{oc}{nh}")
                for cc2 in range(2):
                    pv = pw_sb[cc2][:].rearrange("p (t o) -> p t o", t=2)
                    ov = on_sb[(img, cc2)][:].rearrange("p (t n) -> p t n", t=2)
                    nc.tensor.matmul(
                        p_ps[:],
                        pv[:, :, oc * 128:(oc + 1) * 128],
                        ov[:, :, nh * 512:(nh + 1) * 512],
                        start=(cc2 == 0), stop=False, perf_mode=DRM)
                nc.tensor.matmul(
                    p_ps[:],
                    eye_sb[:],
                    xr_sb[(img, oc)][:, nh * 512:(nh + 1) * 512],
                    start=False, stop=True, skip_group_check=True)
                if nh == 0:
                    yt = sb.tile([128, N], F32, tag="y", bufs=BUFS["y"],
                                 name=f"y{img}{oc}")
                    ytiles[(img, oc)] = yt
                yt = ytiles[(img, oc)]
                nc.scalar.activation(
                    yt[:, nh * 512:(nh + 1) * 512], p_ps[:],
                    mybir.ActivationFunctionType.Copy)
                if nh == 1:
                    nc.gpsimd.dma_start(
                        out=y_h[img, oc * 128:(oc + 1) * 128, :], in_=yt[:])

            ytiles = {}

            # ---------------- attention head machinery --------------------
            def emit_head(img, h, hooks, pend):
                """hooks: {mcp: [callables]} run between slab chains.
                pend: closure finishing the previous head (its last O step,
                normalize and nh1 replay) — run after this head's first two
                slab chains so the previous head's tail overlaps and the PE
                has S work queued ahead of the cross-engine O dependency.
                Returns this head's pend closure."""
                pair, half = h // 2, h % 2
                base = 64 * half
                gh = img * 8 + h
                dve_set = dve_mcs(gh)
                qkr = qk_sb[(img, pair)][base:base + 64].rearrange(
                    "p (b n) -> p b n", b=4)
                es_tiles = []
                o_ref = {}

                def s_exp(mc):
                    mcp, sub = mc // 2, mc % 2
                    if sub == 0:
                        e = sb.tile([128, 2 * N], F8, tag="es",
                                    bufs=BUFS["es"], name=f"es{img}{h}{mcp}")
                        es_tiles.append(e)
                    e = es_tiles[mcp]
                    if mc in dve_set:
                        e8 = e[:].bitcast(I8)
                        for nh in range(2):
                            s_ps = sd.tile([128, 512], F32, tag="sd",
                                           name=f"sd{img}{h}{mc}{nh}")
                            nc.tensor.matmul(
                                s_ps[:],
                                qkr[:, 2:4, mc * 128:(mc + 1) * 128],
                                qkr[:, 0:2, nh * 512:(nh + 1) * 512],
                                start=True, stop=True, perf_mode=DRM)
                            nc.vector.tensor_scalar(
                                e8[:, sub * N + nh * 512:
                                   sub * N + nh * 512 + 512],
                                s_ps[:], A_S, B_S, ALU.mult, ALU.add)
                    else:
                        s_ps = sa.tile([128, N], F32, tag="sa",
                                       name=f"sa{img}{h}{mc}")
                        for nh in range(2):
                            nc.tensor.matmul(
                                s_ps[:, nh * 512:(nh + 1) * 512],
                                qkr[:, 2:4, mc * 128:(mc + 1) * 128],
                                qkr[:, 0:2, nh * 512:(nh + 1) * 512],
                                start=True, stop=True, perf_mode=DRM)
                        nc.scalar.activation(
                            e[:, sub * N:(sub + 1) * N], s_ps[:],
                            mybir.ActivationFunctionType.Exp, scale=SCALE)

                def o_step(nh, mcp, o_ps):
                    vt4 = vt_sb[(img, mcp)][:].rearrange(
                        "p (t h u) -> p t h u", t=2, u=128)
                    esv = es_tiles[mcp][:].rearrange("p (t n) -> p t n", t=2)
                    nc.tensor.matmul(
                        o_ps[:],
                        vt4[:, :, h, :],
                        esv[:, :, nh * 512:(nh + 1) * 512],
                        start=(mcp == 0), stop=(mcp == 3),
                        skip_group_check=True, perf_mode=DRM)

                def divide(nh, o_ps):
                    # DVE tensor_tensor has no divide op and may read only
                    # one non-scalar operand from PSUM: reciprocal the
                    # denominator rows into SBUF, then multiply.
                    cc2, t = h // 4, (h // 2) % 2
                    onr = on_sb[(img, cc2)][:].rearrange(
                        "p (t n) -> p t n", t=2)
                    dn = sb.tile([64, 512], F32, tag="dn", bufs=3,
                                 name=f"dn{img}{h}{nh}")
                    nc.vector.reciprocal(dn[:], o_ps[64:128, :])
                    nc.vector.tensor_mul(
                        onr[base:base + 64, t, nh * 512:(nh + 1) * 512],
                        o_ps[0:64, :], dn[:])

                for mcp in range(4):
                    s_exp(2 * mcp)
                    s_exp(2 * mcp + 1)
                    if mcp == 1:
                        if pend is not None:
                            pend()
                        o_ref["o0"] = po.tile([128, 512], F32, tag="o",
                                              name=f"o{img}{h}n0")
                    for fn in hooks.get(mcp, ()):
                        fn()
                    if mcp >= 1:
                        o_step(0, mcp - 1, o_ref["o0"])

                def my_pend():
                    o_step(0, 3, o_ref["o0"])
                    divide(0, o_ref["o0"])
                    o1 = po.tile([128, 512], F32, tag="o", name=f"o{img}{h}n1")
                    for mcp in range(4):
                        o_step(1, mcp, o1)
                    divide(1, o1)

                return my_pend

            # ---------------- emission schedule ----------------------------
            warm = wp.tile([1, 1], F32, tag="warm", name="warm")
            nc.vector.memset(warm[:], 0.0)
            nc.scalar.activation(
                warm[:], warm[:], mybir.ActivationFunctionType.Exp)

            bqk_sb, bv_sb, eye_sb = load_weights_and_x0()
            alloc_qk(0)
            alloc_on(0)
            for u in (lambda: mm_qk(0, 0, 0), lambda: mm_qk(0, 0, 1),
                      lambda: mm_qk(0, 4, 0), lambda: mm_qk(0, 4, 1),
                      lambda: mm_v(0, 0), lambda: mm_v(0, 1)):
                u()

            # filler work queue, drained between slab chains
            WQ = []

            def q_qk(img, oc):
                WQ.extend([lambda nh=nh: mm_qk(img, oc, nh) for nh in range(2)])

            def q_v(img, mc):
                WQ.append(lambda: mm_v(img, mc))

            q_v(0, 2); q_v(0, 3)
            q_v(0, 4); q_v(0, 5)
            q_v(0, 6); q_v(0, 7)
            q_qk(0, 1); q_qk(0, 5)
            q_qk(0, 2); q_qk(0, 6)
            q_qk(0, 3); q_qk(0, 7)
            WQ.append(lambda: load_xq(1))
            WQ.append(lambda: alloc_qk(1))
            q_qk(1, 0); q_qk(1, 4)
            q_v(1, 0); q_v(1, 1)
            q_v(1, 2); q_v(1, 3)
            q_qk(1, 1); q_qk(1, 5)
            q_v(1, 4); q_v(1, 5)
            q_v(1, 6); q_v(1, 7)
            q_qk(1, 2); q_qk(1, 6)
            q_qk(1, 3); q_qk(1, 7)
            WQ.append(lambda: alloc_on(1))
            WQ.append(lambda: load_xr(0))

            def drain(k):
                fns = []
                while WQ and len(fns) < k:
                    fns.append(WQ.pop(0))
                return fns

            pend = None
            for img in range(2):
                if img == 1:
                    for oc in range(4):
                        for nh in range(2):
                            WQ.append(lambda oc=oc, nh=nh: mm_proj(0, oc, nh))
                    WQ.append(lambda: load_xr(1))
                for h in range(8):
                    hooks = {mcp: drain(2) for mcp in (1, 2, 3)}
                    pend = emit_head(img, h, hooks, pend)
            pend()
            for fn in WQ:
                fn()
            for oc in range(4):
                for nh in range(2):
                    mm_proj(1, oc, nh)

    _split_multi_waits(nc)
    return nc


_CACHE = {}


def _get_nc(mode=MM_MODE):
    if "nc" not in _CACHE:
        _CACHE["nc"] = build_nc()
    return _CACHE["nc"]


def prepare_inputs(x, qkv_w, qkv_b, proj_w, proj_b, mode=MM_MODE):
    x = np.asarray(x, np.float32).reshape(B, C, N)
    qkv_w = np.asarray(qkv_w, np.float32)
    qkv_b = np.asarray(qkv_b, np.float32)
    proj_w = np.asarray(proj_w, np.float32)
    proj_b = np.asarray(proj_b, np.float32)

    # channel c = cc2*256 + t*128 + p  ->  [cc2, p, (t, inner)]
    xq = np.ascontiguousarray(
        x.astype(NP8).reshape(B, 2, 2, 128, N).transpose(0, 1, 3, 2, 4)
        .reshape(B, 2, 128, 2 * N))
    wqkv = np.ascontiguousarray(
        qkv_w.T.astype(NP8).reshape(2, 2, 128, 3 * C).transpose(0, 2, 1, 3)
        .reshape(2, 128, 2 * 1536))
    pw = np.ascontiguousarray(
        proj_w.T.astype(NP8).reshape(2, 2, 128, C).transpose(0, 2, 1, 3)
        .reshape(2, 128, 2 * 512))
    xr = np.ascontiguousarray(x + proj_b[None, :, None])
    bqk = np.ascontiguousarray(qkv_b[:1024].reshape(8, 128).T)
    bv = np.ascontiguousarray(np.broadcast_to(qkv_b[2 * C:], (128, C)))
    zz = np.zeros((1, N), NP8)
    oo = np.ones((1, N), NP8)
    eye = np.eye(128, dtype=np.float32)

    in_maps = []
    for c in range(NCORES):
        sl = slice(c * BPC, (c + 1) * BPC)
        in_maps.append({
            "xq": xq[sl], "xr": xr[sl], "wqkv": wqkv, "pw": pw,
            "bqk": bqk, "bv": bv, "zz": zz, "oo": oo, "eye": eye,
        })
    return in_maps


def run(x, qkv_w, qkv_b, proj_w, proj_b, mode=MM_MODE, **spmd_kwargs):
    nc = _get_nc(mode)
    in_maps = prepare_inputs(x, qkv_w, qkv_b, proj_w, proj_b, mode)
    res = run_bass_kernel_spmd(nc, in_maps, list(range(NCORES)), **spmd_kwargs)
    y = np.concatenate([np.asarray(res.results[c]["y"]) for c in range(NCORES)],
                       axis=0)
    return res, y.reshape(B, C, 32, 32).astype(np.float32)


def kernel(x, qkv_w, qkv_b, proj_w, proj_b):
    _, y = run(x, qkv_w, qkv_b, proj_w, proj_b)
    return y
